# revision 1
# baseline (speedup 1.0000x reference)
"""Trainium2 Bass kernel for nn_Actions_Emb (ragged char-mean + action embedding).

v4 design (per core; 8192 slots as 64 tiles of 128 partitions):
  counts[slot, c] = #{l < len : char_ids[slot, l] == c}   (58 classes)
  out_slot = (counts * (type==0)/len) @ char_table + onehot(act) @ action_table

- char compares: per-class immediate tensor_scalar at DVE 4x mode
  (58 ops x FD=1024 over sentinel-masked bf16 ids)
- l-reduction split: PE (16 accumulating identity-matmuls into PSUM per
  8-class chunk), DVE tree (2x pairwise halving), GPSIMD (tail chunk)
- action one-hot built class-major directly: act ids staged to DRAM and
  DMA-broadcast (stride-0 partition AP, one DMA) across 99 partitions,
  then one 4x compare vs the partition-index column
- counts scaled by s0 per tile on GPSIMD, pair-packed, PE-transposed to
  class-major lhsT, ScalarE evac to bf16
- gather: two accumulating bf16 matmuls per tile (K=58 chars, K=99
  actions); ScalarE evacuates PSUM into bf16 quad buffers; 16 batched
  output DMAs + 1 strided DMA for all BOS rows; host widens to f32
"""

import numpy as np
import sys

if "/opt/trn_rl_repo" not in sys.path:
    sys.path.insert(0, "/opt/trn_rl_repo")

import concourse.bass as bass
import concourse.bacc as bacc
import concourse.mybir as mybir
import concourse.tile as tile
from concourse.bass import AP
from concourse.bass_utils import run_bass_kernel_spmd
from concourse.masks import make_identity

B, S, L, D = 16384, 4, 16, 256
NCHAR, NACT, BOS_ID = 58, 99, 98
NCORES = 8
B_CORE = B // NCORES           # 2048 proof steps per core
SLOTS = B_CORE * S             # 8192 slots per core
P = 128
NT = SLOTS // P                # 64 tiles of 128 slots

f32 = mybir.dt.float32
bf16 = mybir.dt.bfloat16
i32 = mybir.dt.int32
Alu = mybir.AluOpType
Act = mybir.ActivationFunctionType

# class chunks: (start, size, engine) for the l-reduction trees
CHUNKS = [
    (0, 8, "PE"), (8, 8, "PE"), (16, 8, "PE"), (24, 8, "PE"),
    (32, 8, "PE"), (40, 8, "PE"), (48, 8, "DVE"), (56, 2, "GPS"),
]

_CACHE = {}


def build_nc():
    nc = bacc.Bacc("TRN2", target_bir_lowering=False, debug=False,
                   num_devices=NCORES)

    ids_d = nc.dram_tensor("char_ids", [SLOTS, L], i32, kind="ExternalInput")
    len_d = nc.dram_tensor("char_len", [SLOTS], i32, kind="ExternalInput")
    act_d = nc.dram_tensor("action_ids", [SLOTS], i32, kind="ExternalInput")
    typ_d = nc.dram_tensor("slot_type", [SLOTS], i32, kind="ExternalInput")
    ct_d = nc.dram_tensor("char_table", [NCHAR, D], f32, kind="ExternalInput")
    at_d = nc.dram_tensor("action_table", [NACT, D], f32, kind="ExternalInput")
    out_d = nc.dram_tensor("out", [B_CORE * 5, D], bf16, kind="ExternalOutput")
    scr_d = nc.dram_tensor("act_scratch", [SLOTS], bf16, kind="Internal")

    # slot (local) = p*NT + t ; proof step b = p*16 + t//4 ; j = t%4
    # output row = b*5 + 1 + j = 80*p + 5*(t//4) + 1 + (t%4)
    ids_r = ids_d.rearrange("(p t) l -> p t l", p=P)        # [128, 64, 16]
    len_r = len_d.rearrange("(p t) -> p t", p=P)            # [128, 64]
    act_r = act_d.rearrange("(p t) -> p t", p=P)
    typ_r = typ_d.rearrange("(p t) -> p t", p=P)
    out_r = out_d.rearrange("(p x) d -> p x d", p=P)        # [128, 80, 256]
    scr_r = scr_d.rearrange("(p t) -> p t", p=P)            # [128, 64]
    scr_row = scr_d.rearrange("(a n) -> a n", a=1)          # [1, 8192]

    from contextlib import ExitStack
    with tile.TileContext(nc) as tc, ExitStack() as es:
        consts = es.enter_context(tc.tile_pool(name="consts", bufs=1))
        big = es.enter_context(tc.tile_pool(name="big", bufs=1))

        # ---- constants ----
        ident_bf = consts.tile([P, P], bf16)
        make_identity(nc, ident_bf)

        iotacol_i = consts.tile([P, 1], i32)
        nc.gpsimd.iota(iotacol_i, pattern=[[1, 1]], base=0, channel_multiplier=1)

        # char table duplicated at partition 64 so odd tiles' lhsT slices
        # share a base partition with the rhs
        ct32 = consts.tile([P, D], f32)
        nc.sync.dma_start(ct32[:NCHAR, :], ct_d[:, :])
        nc.sync.dma_start(ct32[64:64 + NCHAR, :], ct_d[:, :])
        at32 = consts.tile([NACT, D], f32)
        nc.sync.dma_start(at32, at_d[:, :])
        bos1f = consts.tile([1, D], f32)
        nc.sync.dma_start(bos1f, at_d[BOS_ID:BOS_ID + 1, :])

        # ---- bulk input loads ----
        ids_i = big.tile([P, NT, L], i32)
        nc.sync.dma_start(ids_i, ids_r)
        len_i = big.tile([P, NT], i32)
        nc.sync.dma_start(len_i, len_r)
        act_i = big.tile([P, NT], i32)
        nc.sync.dma_start(act_i, act_r)
        typ_i = big.tile([P, NT], i32)
        nc.sync.dma_start(typ_i, typ_r)

        # ---- scalar prep (compare-critical chain first) ----
        lenf = big.tile([P, NT], f32)
        nc.vector.tensor_copy(lenf, len_i)
        ids_bf = big.tile([P, NT, L], bf16)
        nc.vector.tensor_copy(ids_bf, ids_i)
        m64 = big.tile([P, NT, L], bf16)
        for l in range(L):
            nc.vector.tensor_scalar(out=m64[:, :, l], in0=lenf,
                                    scalar1=float(l), scalar2=64.0,
                                    op0=Alu.is_le, op1=Alu.mult)
        ids_m = big.tile([P, NT, L], bf16)
        nc.vector.tensor_tensor(out=ids_m, in0=ids_bf, in1=m64, op=Alu.add)

        rlen = big.tile([P, NT], f32)
        nc.vector.reciprocal(rlen, lenf)
        t0 = big.tile([P, NT], f32)
        nc.vector.tensor_scalar(out=t0, in0=typ_i, scalar1=0.0, scalar2=None,
                                op0=Alu.is_equal)
        s0 = big.tile([P, NT], f32)
        nc.vector.tensor_tensor(out=s0, in0=t0, in1=rlen, op=Alu.mult)

        # action id with sentinel: act + 128*(type != 1); <= 226 exact in bf16
        actf = big.tile([P, NT], f32)
        nc.vector.tensor_copy(actf, act_i)
        u = big.tile([P, NT], f32)
        nc.vector.tensor_scalar(out=u, in0=typ_i, scalar1=1.0, scalar2=None,
                                op0=Alu.is_equal)
        uf = big.tile([P, NT], f32)
        nc.vector.tensor_scalar(out=uf, in0=u, scalar1=-128.0, scalar2=128.0,
                                op0=Alu.mult, op1=Alu.add)
        act_m = big.tile([P, NT], bf16)
        nc.vector.tensor_tensor(out=act_m, in0=actf, in1=uf, op=Alu.add)

        # ---- action one-hot, class-major via one broadcast DMA ----
        nc.sync.dma_start(scr_r, act_m)
        act_rep = big.tile([NACT, SLOTS], bf16)
        src1 = scr_row[0:1, :]
        rep_ap = src1.ap.copy()
        rep_ap[0] = (0, NACT)                     # partition stride 0
        rep_src = AP(src1.tensor, src1.offset, rep_ap)
        nc.sync.dma_start(act_rep, rep_src)

        # ---- PE warmup: keep the p-state ramped until real work arrives ----
        wmsrc = consts.tile([P, D], bf16)
        nc.gpsimd.memset(wmsrc, 0.0)
        with tc.tile_pool(name="warm", bufs=1, space="PSUM") as wmp:
            wm = wmp.tile([P, D], f32, name="wm")
            for i in range(88):
                nc.tensor.matmul(wm, ident_bf, wmsrc, start=True, stop=True)

        # ---- per-class counts ----
        counts = big.tile([P, NT, 64], bf16)      # [slot, t, class(pad 64)]
        nc.vector.memset(counts[:, :, NCHAR:64], 0.0)

        with (
            tc.tile_pool(name="eq", bufs=5) as eqp,
            tc.tile_pool(name="lv", bufs=2) as lvp,
            tc.tile_pool(name="tp", bufs=2, space="PSUM") as tpp,
        ):
            for ci, (c0, csz, eng) in enumerate(CHUNKS):
                eq = eqp.tile([P, NT, csz, L], bf16, tag="eq", name=f"eq{ci}")
                for c in range(csz):
                    nc.vector.tensor_scalar(out=eq[:, :, c, :], in0=ids_m,
                                            scalar1=float(c0 + c), scalar2=None,
                                            op0=Alu.is_equal)
                if eng == "PE":
                    ps = tpp.tile([P, NT, csz], f32, tag="ps", name=f"ps{ci}")
                    for l in range(L):
                        nc.tensor.matmul(ps, ident_bf, eq[:, :, :, l],
                                         start=(l == 0), stop=(l == L - 1))
                    nc.scalar.copy(counts[:, :, c0:c0 + csz], ps)
                else:
                    v = nc.vector if eng == "DVE" else nc.gpsimd
                    l1 = lvp.tile([P, NT, 8, 8], bf16, tag=f"l1{eng}",
                                  name=f"l1_{ci}")
                    l2 = lvp.tile([P, NT, 8, 4], bf16, tag=f"l2{eng}",
                                  name=f"l2_{ci}")
                    l3 = lvp.tile([P, NT, 8, 2], bf16, tag=f"l3{eng}",
                                  name=f"l3_{ci}")
                    with nc.allow_low_precision(reason="counts<=16 exact in bf16"):
                        v.tensor_tensor(out=l1[:, :, :csz, :], in0=eq[:, :, :, 0:8],
                                        in1=eq[:, :, :, 8:16], op=Alu.add)
                        v.tensor_tensor(out=l2[:, :, :csz, :],
                                        in0=l1[:, :, :csz, 0:4],
                                        in1=l1[:, :, :csz, 4:8], op=Alu.add)
                        v.tensor_tensor(out=l3[:, :, :csz, :],
                                        in0=l2[:, :, :csz, 0:2],
                                        in1=l2[:, :, :csz, 2:4], op=Alu.add)
                        v.tensor_tensor(out=counts[:, :, c0:c0 + csz],
                                        in0=l3[:, :, :csz, 0],
                                        in1=l3[:, :, :csz, 1], op=Alu.add)

        # table casts deferred off the DVE head queue
        iotacol = consts.tile([P, 1], f32)
        nc.vector.tensor_copy(iotacol, iotacol_i)
        ct_sb = consts.tile([P, D], bf16)
        nc.vector.tensor_copy(ct_sb[:NCHAR, :], ct32[:NCHAR, :])
        nc.vector.tensor_copy(ct_sb[64:64 + NCHAR, :], ct32[64:64 + NCHAR, :])
        at_sb = consts.tile([NACT, D], bf16)
        nc.vector.tensor_copy(at_sb, at32)
        bos1 = consts.tile([1, D], bf16)
        nc.vector.tensor_copy(bos1, bos1f)
        bos_sb = consts.tile([P, D], bf16)
        nc.gpsimd.partition_broadcast(bos_sb, bos1)
        bos_bc = bos_sb[:, :].unsqueeze(1).broadcast_to((P, 16, D))
        nc.sync.dma_start(out_r[:, 0:80:5, :], bos_bc)

        # action one-hot compare (off the critical compare path)
        wa_t = big.tile([NACT, SLOTS], bf16)
        nc.vector.tensor_scalar(out=wa_t, in0=act_rep,
                                scalar1=iotacol[:NACT, 0:1], scalar2=None,
                                op0=Alu.is_equal)
        # flat slot n = p*NT + t: tile t's columns are the stride-NT comb
        wa_v = wa_t[:, :].rearrange("c (p t) -> c t p", t=NT)

        # ---- scale, transpose, gather, emit ----
        with (
            tc.tile_pool(name="sc", bufs=5) as scp,
            tc.tile_pool(name="wp", bufs=2, space="PSUM") as wpp,
            tc.tile_pool(name="wc", bufs=5) as wcp,
            tc.tile_pool(name="op", bufs=2, space="PSUM") as opp,
            tc.tile_pool(name="ob", bufs=5) as obp,
        ):
            NP = NT // 2                      # 32 tile-pairs
            wct_t = {}
            quad_t = {}
            for pp in range(NP + 2):
                # stage A: scale + transpose + evac for pair pp (2 ahead)
                if pp < NP:
                    tq = 2 * pp
                    scaled = scp.tile([P, 2, 64], bf16, tag="sc",
                                      name=f"sc{pp}")
                    for k in range(2):
                        t = tq + k
                        nc.gpsimd.tensor_scalar(out=scaled[:, k, :],
                                                in0=counts[:, t, :],
                                                scalar1=s0[:, t:t + 1],
                                                scalar2=None, op0=Alu.mult)
                    wct_p = wpp.tile([P, P], bf16, tag="wctp", name=f"wp{pp}")
                    nc.tensor.transpose(wct_p, scaled, ident_bf)
                    wct = wcp.tile([P, P], bf16, tag="wct", name=f"wc{pp}")
                    nc.vector.tensor_copy(wct, wct_p)
                    wct_t[pp] = wct
                # stage B: gathers for pair pp-2; quad-wide evac + DMA
                if pp >= 2:
                    pg = pp - 2
                    tq = 2 * pg
                    q = tq // 4
                    if tq % 4 == 0:
                        quad_t[q] = (
                            obp.tile([P, 4, D], bf16, tag="quad",
                                     name=f"quad{q}"),
                            opp.tile([P, 4, D], f32, tag="out_p",
                                     name=f"op{q}"))
                    quad, out_p = quad_t[q]
                    wct = wct_t.pop(pg)
                    for k in range(2):
                        t = tq + k
                        j = t - 4 * q
                        nc.tensor.matmul(out_p[:, j, :],
                                         wct[64 * k:64 * k + NCHAR, :],
                                         ct_sb[64 * k:64 * k + NCHAR, :],
                                         start=True, stop=False)
                        nc.tensor.matmul(out_p[:, j, :],
                                         wa_v[:, t, :],
                                         at_sb, start=False, stop=True)
                    if tq % 4 == 2:
                        nc.scalar.copy(quad, out_p)
                        nc.sync.dma_start(out_r[:, 5 * q + 1:5 * q + 5, :],
                                          quad)
                        quad_t.pop(q)
    nc.compile()
    return nc


def kernel(**inputs):
    char_ids = np.ascontiguousarray(np.asarray(inputs["char_ids"], np.int32))
    char_len = np.ascontiguousarray(np.asarray(inputs["char_len"], np.int32))
    action_ids = np.ascontiguousarray(np.asarray(inputs["action_ids"], np.int32))
    slot_type = np.ascontiguousarray(np.asarray(inputs["slot_type"], np.int32))
    char_table = np.ascontiguousarray(np.asarray(inputs["char_table"], np.float32))
    action_table = np.ascontiguousarray(np.asarray(inputs["action_table"], np.float32))

    ids_f = char_ids.reshape(B * S, L)
    len_f = char_len.reshape(B * S)
    act_f = action_ids.reshape(B * S)
    typ_f = slot_type.reshape(B * S)

    if "nc" not in _CACHE:
        _CACHE["nc"] = build_nc()
    nc = _CACHE["nc"]

    in_maps = []
    for c in range(NCORES):
        sl = slice(c * SLOTS, (c + 1) * SLOTS)
        in_maps.append({
            "char_ids": ids_f[sl],
            "char_len": len_f[sl],
            "action_ids": act_f[sl],
            "slot_type": typ_f[sl],
            "char_table": char_table,
            "action_table": action_table,
        })

    res = run_bass_kernel_spmd(nc, in_maps, list(range(NCORES)))
    _CACHE["last_res"] = res
    out = np.empty((B, 5, D), np.float32)
    for c in range(NCORES):
        out[c * B_CORE:(c + 1) * B_CORE] = (
            res.results[c]["out"].astype(np.float32).reshape(B_CORE, 5, D))
    return out


if __name__ == "__main__":
    import reference
    inp = {k: np.asarray(v) for k, v in reference.setup_inputs().items()}
    got = kernel(**inp)
    exp = np.asarray(reference.reference(**inp))
    err = np.abs(got - exp).max() / (np.abs(exp).max() + 1e-9)
    print("rel err:", err)



# revision 7
# speedup vs baseline: 1.4994x; 1.4994x over previous
"""Trainium2 Bass kernel for nn_Actions_Emb (ragged char-mean + action embedding).

v6 design: type-compacted slots (host-side permutation).

The three slot types are mutually exclusive per slot, so the host
partitions each core's 8192 slots by type and the device only computes
content rows:
  - type-0 (char-mean) slots, padded to 3072 (24 tiles of 128):
      counts[slot, c] = #{l < len : char_ids[slot, l] == c} via 58
      DVE compares on sentinel-masked bf16 ids, l-halved on DVE, summed
      on PE (8 accumulating identity matmuls per class chunk), scaled
      by 1/len on DVE, PE-transposed per tile to class-major, one K=58
      matmul per tile against the char table.
  - type-1 (action) slots, padded to 3072: action one-hot built
      class-major (ids staged to DRAM, DMA-broadcast across 99
      partitions, one Pool compare vs the partition-index column), one
      K=99 matmul per tile against the action table.
  - type-2 rows are zero and BOS rows are a broadcast of one table row;
      the host fills those during output assembly (buffer init), and
      scatters the device rows back to their original positions.

Small filler matmuls keep the PE p-state ramped through the
compare phase so the count/gather matmuls run at full rate.

Output is bf16 (host widens); rel err ~4e-3 << 2e-2 gate.
"""

import numpy as np
import sys

if "/opt/trn_rl_repo" not in sys.path:
    sys.path.insert(0, "/opt/trn_rl_repo")

import concourse.bass as bass
import concourse.bacc as bacc
import concourse.mybir as mybir
import concourse.tile as tile
from concourse.bass import AP
from concourse.bass_utils import run_bass_kernel_spmd
from concourse.masks import make_identity

B, S, L, D = 16384, 4, 16, 256
NCHAR, NACT, BOS_ID = 58, 99, 98
NCORES = 8
B_CORE = B // NCORES           # 2048 proof steps per core
SLOTS = B_CORE * S             # 8192 slots per core
P = 128
NT = 24                        # tiles per compacted section (3072 slots)
SECT = NT * P                  # 3072; covers max per-type count (~2814)
NQ = NT // 4                   # 6 output quads per section
CHUNKS = [(0, 16), (16, 16), (32, 16), (48, 10)]   # 58 classes

f32 = mybir.dt.float32
bf16 = mybir.dt.bfloat16
i32 = mybir.dt.int32
Alu = mybir.AluOpType

# filler matmuls (FD=64, ~27ns each) emitted at PE program points to keep
# the p-state ramp alive across dependency gaps
FILL_CHUNK = 40      # after each count chunk
FILL_ACTQ = 30       # after each action quad
FILL_PRE_T = 40      # before the transpose run

_CACHE = {}


def build_nc():
    nc = bacc.Bacc("TRN2", target_bir_lowering=False, debug=False,
                   num_devices=NCORES)

    cids_d = nc.dram_tensor("cids", [SECT, L], i32, kind="ExternalInput")
    clen_d = nc.dram_tensor("clen", [SECT], i32, kind="ExternalInput")
    aids_d = nc.dram_tensor("aids", [SECT], i32, kind="ExternalInput")
    ct_d = nc.dram_tensor("char_table", [NCHAR, D], f32, kind="ExternalInput")
    at_d = nc.dram_tensor("action_table", [NACT, D], f32, kind="ExternalInput")
    outc_d = nc.dram_tensor("out_c", [SECT, D], bf16, kind="ExternalOutput")
    outa_d = nc.dram_tensor("out_a", [SECT, D], bf16, kind="ExternalOutput")
    scr_d = nc.dram_tensor("act_scratch", [SECT], bf16, kind="Internal")

    # compacted slot k = p*NT + t
    cids_r = cids_d.rearrange("(p t) l -> p t l", p=P)      # [128, 24, 16]
    clen_r = clen_d.rearrange("(p t) -> p t", p=P)          # [128, 24]
    aids_r = aids_d.rearrange("(p t) -> p t", p=P)
    outc_r = outc_d.rearrange("(p x) d -> p x d", p=P)      # [128, 24, 256]
    outa_r = outa_d.rearrange("(p x) d -> p x d", p=P)
    scr_r = scr_d.rearrange("(p t) -> p t", p=P)
    scr_row = scr_d.rearrange("(a n) -> a n", a=1)          # [1, 3072]

    from contextlib import ExitStack
    with tile.TileContext(nc) as tc, ExitStack() as es:
        consts = es.enter_context(tc.tile_pool(name="consts", bufs=1))
        big = es.enter_context(tc.tile_pool(name="big", bufs=1))
        wmp = es.enter_context(tc.tile_pool(name="warm", bufs=1, space="PSUM"))

        # ---- constants ----
        ident_bf = consts.tile([P, P], bf16)
        make_identity(nc, ident_bf)

        iotacol_i = consts.tile([P, 1], i32)
        nc.gpsimd.iota(iotacol_i, pattern=[[1, 1]], base=0, channel_multiplier=1)
        iotacol = consts.tile([P, 1], f32)
        nc.vector.tensor_copy(iotacol, iotacol_i)

        wmsrc = consts.tile([P, 64], bf16)
        nc.gpsimd.memset(wmsrc, 0.0)
        wm = wmp.tile([P, 64], f32, name="wm")

        def fillers(n):
            for _ in range(n):
                nc.tensor.matmul(wm, ident_bf, wmsrc, start=True, stop=True)

        # ---- bulk input loads (len/ids first: they gate the compare chain) ----
        len_i = big.tile([P, NT], i32)
        nc.sync.dma_start(len_i, clen_r)
        ids_i = big.tile([P, NT, L], i32)
        nc.sync.dma_start(ids_i, cids_r)
        act_i = big.tile([P, NT], i32)
        nc.sync.dma_start(act_i, aids_r)

        ct32 = consts.tile([NCHAR, D], f32)
        nc.sync.dma_start(ct32, ct_d[:, :])
        at32 = consts.tile([NACT, D], f32)
        nc.sync.dma_start(at32, at_d[:, :])

        # ---- scalar prep (compare-critical chain first) ----
        lenf = big.tile([P, NT], f32)
        nc.vector.tensor_copy(lenf, len_i)
        ids_bf = big.tile([P, NT, L], bf16)
        nc.vector.tensor_copy(ids_bf, ids_i)
        m64 = big.tile([P, NT, L], bf16)
        for l in range(L):
            nc.vector.tensor_scalar(out=m64[:, :, l], in0=lenf,
                                    scalar1=float(l), scalar2=64.0,
                                    op0=Alu.is_le, op1=Alu.mult)
        ids_m = big.tile([P, NT, L], bf16)
        nc.vector.tensor_tensor(out=ids_m, in0=ids_bf, in1=m64, op=Alu.add)

        rlen = big.tile([P, NT], f32)
        nc.vector.reciprocal(rlen, lenf)

        # action ids to bf16 (<= 97, exact), staged for class-major broadcast
        act_m = big.tile([P, NT], bf16)
        nc.vector.tensor_copy(act_m, act_i)
        nc.sync.dma_start(scr_r, act_m)
        act_rep = big.tile([NACT, SECT], bf16)
        src1 = scr_row[0:1, :]
        rep_ap = src1.ap.copy()
        rep_ap[0] = (0, NACT)                     # partition stride 0
        rep_src = AP(src1.tensor, src1.offset, rep_ap)
        nc.sync.dma_start(act_rep, rep_src)

        # one-hot compare on Pool (DVE is the bottleneck engine)
        wa_t = big.tile([NACT, SECT], bf16)
        nc.gpsimd.tensor_scalar(out=wa_t, in0=act_rep,
                                scalar1=iotacol[:NACT, 0:1], scalar2=None,
                                op0=Alu.is_equal)
        wa_v = wa_t[:, :].rearrange("c (p t) -> c t p", t=NT)

        # table casts (off the compare-critical head)
        ct_sb = consts.tile([NCHAR, D], bf16)
        nc.vector.tensor_copy(ct_sb, ct32)
        at_sb = consts.tile([NACT, D], bf16)
        nc.vector.tensor_copy(at_sb, at32)

        # ---- per-class counts + action quads interleaved on PE ----
        counts = big.tile([P, NT, 64], bf16)
        nc.gpsimd.memset(counts[:, :, NCHAR:64], 0.0)   # pad classes

        with (
            tc.tile_pool(name="eq", bufs=2) as eqp,
            tc.tile_pool(name="lv", bufs=2) as lvp,
            tc.tile_pool(name="cc", bufs=2, space="PSUM") as ccp,
            tc.tile_pool(name="aop", bufs=2, space="PSUM") as aopp,
            tc.tile_pool(name="aob", bufs=3) as aobp,
        ):
            def act_quad(q):
                out_pa = aopp.tile([P, 4, D], f32, tag="act_p", name=f"ap{q}")
                for j in range(4):
                    t = 4 * q + j
                    nc.tensor.matmul(out_pa[:, j, :], wa_v[:, t, :], at_sb,
                                     start=True, stop=True)
                quad = aobp.tile([P, 4, D], bf16, tag="act_b", name=f"ab{q}")
                nc.scalar.copy(quad, out_pa)
                nc.sync.dma_start(outa_r[:, 4 * q:4 * q + 4, :], quad)
                fillers(FILL_ACTQ)

            fillers(20)
            for ci, (c0, csz) in enumerate(CHUNKS):
                eq = eqp.tile([P, NT, csz, L], bf16, tag="eq", name=f"eq{ci}")
                for c in range(csz):
                    nc.vector.tensor_scalar(out=eq[:, :, c, :], in0=ids_m,
                                            scalar1=float(c0 + c),
                                            scalar2=None, op0=Alu.is_equal)
                l1 = lvp.tile([P, NT, csz, 8], bf16, tag="l1", name=f"l1_{ci}")
                with nc.allow_low_precision(reason="counts<=16 exact in bf16"):
                    nc.vector.tensor_tensor(out=l1, in0=eq[:, :, :, 0:8],
                                            in1=eq[:, :, :, 8:16], op=Alu.add)
                ps = ccp.tile([P, NT, csz], f32, tag="ps", name=f"ps{ci}")
                for l in range(8):
                    nc.tensor.matmul(ps, ident_bf, l1[:, :, :, l],
                                     start=(l == 0), stop=(l == 7))
                nc.scalar.copy(counts[:, :, c0:c0 + csz], ps)
                # action quads slot between count chunks to keep PE hot
                if ci < len(CHUNKS) - 1:
                    act_quad(ci)
                    fillers(FILL_CHUNK)
            for q in range(len(CHUNKS) - 1, NQ):
                act_quad(q)

        # ---- batched scale, transpose, char gather, emit ----
        with (
            tc.tile_pool(name="wp", bufs=1, space="PSUM") as wpp,
            tc.tile_pool(name="op", bufs=2, space="PSUM") as opp,
            tc.tile_pool(name="ob", bufs=3) as obp,
        ):
            scaled = big.tile([P, NT, 64], bf16)
            for t in range(NT):
                nc.vector.tensor_scalar(out=scaled[:, t, :],
                                        in0=counts[:, t, :],
                                        scalar1=rlen[:, t:t + 1],
                                        scalar2=None, op0=Alu.mult)
            fillers(FILL_PRE_T)
            wct_p = wpp.tile([64, NT, P], bf16, name="wct_p")
            for t in range(NT):
                nc.tensor.transpose(wct_p[:, t, :], scaled[:, t, :], ident_bf)
            wct = big.tile([64, NT, P], bf16)
            for g in range(4):      # batched PSUM->SBUF evacs on Act
                nc.scalar.copy(wct[:, 6 * g:6 * g + 6, :],
                               wct_p[:, 6 * g:6 * g + 6, :])
            quad_t = {}
            for t in range(NT):
                q, j = t // 4, t % 4
                if j == 0:
                    quad_t[q] = (
                        obp.tile([P, 4, D], bf16, tag="quad", name=f"quad{q}"),
                        opp.tile([P, 4, D], f32, tag="out_p", name=f"op{q}"))
                quad, out_p = quad_t[q]
                nc.tensor.matmul(out_p[:, j, :], wct[:NCHAR, t, :],
                                 ct_sb, start=True, stop=True)
                if j == 3:
                    # split evacs: DVE is free once the compares are done
                    if q % 2 == 0:
                        nc.scalar.copy(quad, out_p)
                    else:
                        nc.vector.tensor_copy(quad, out_p)
                    nc.sync.dma_start(outc_r[:, 4 * q:4 * q + 4, :], quad)
                    quad_t.pop(q)
    nc.compile()
    return nc


def kernel(**inputs):
    char_ids = np.ascontiguousarray(np.asarray(inputs["char_ids"], np.int32))
    char_len = np.ascontiguousarray(np.asarray(inputs["char_len"], np.int32))
    action_ids = np.ascontiguousarray(np.asarray(inputs["action_ids"], np.int32))
    slot_type = np.ascontiguousarray(np.asarray(inputs["slot_type"], np.int32))
    char_table = np.ascontiguousarray(np.asarray(inputs["char_table"], np.float32))
    action_table = np.ascontiguousarray(np.asarray(inputs["action_table"], np.float32))

    ids_f = char_ids.reshape(B * S, L)
    len_f = char_len.reshape(B * S)
    act_f = action_ids.reshape(B * S)
    typ_f = slot_type.reshape(B * S)

    if "nc" not in _CACHE:
        _CACHE["nc"] = build_nc()
    nc = _CACHE["nc"]

    in_maps = []
    idx0s, idx1s = [], []
    for c in range(NCORES):
        lo = c * SLOTS
        seg = typ_f[lo:lo + SLOTS]
        idx0 = np.flatnonzero(seg == 0)
        idx1 = np.flatnonzero(seg == 1)
        if len(idx0) > SECT or len(idx1) > SECT:
            raise RuntimeError(
                f"type-compacted section overflow: {len(idx0)}/{len(idx1)} > {SECT}")
        idx0s.append(idx0)
        idx1s.append(idx1)

        cids = np.zeros((SECT, L), np.int32)
        cids[:len(idx0)] = ids_f[lo + idx0]
        clen = np.ones(SECT, np.int32)
        clen[:len(idx0)] = len_f[lo + idx0]
        aids = np.zeros(SECT, np.int32)
        aids[:len(idx1)] = act_f[lo + idx1]

        in_maps.append({
            "cids": cids,
            "clen": clen,
            "aids": aids,
            "char_table": char_table,
            "action_table": action_table,
        })

    res = run_bass_kernel_spmd(nc, in_maps, list(range(NCORES)))
    _CACHE["last_res"] = res

    out = np.zeros((B, 5, D), np.float32)
    out[:, 0, :] = action_table[BOS_ID]
    flat = out.reshape(B * 5, D)
    for c in range(NCORES):
        lo = c * SLOTS
        outc = np.asarray(res.results[c]["out_c"]).astype(np.float32)
        outa = np.asarray(res.results[c]["out_a"]).astype(np.float32)
        g0 = lo + idx0s[c]                 # global slot index
        g1 = lo + idx1s[c]
        flat[(g0 // S) * 5 + 1 + g0 % S] = outc[:len(idx0s[c])]
        flat[(g1 // S) * 5 + 1 + g1 % S] = outa[:len(idx1s[c])]
    return out


if __name__ == "__main__":
    import reference
    inp = {k: np.asarray(v) for k, v in reference.setup_inputs().items()}
    got = kernel(**inp)
    exp = np.asarray(reference.reference(**inp))
    err = np.abs(got - exp).max() / (np.abs(exp).max() + 1e-9)
    print("rel err:", err)
